# revision 20
# baseline (speedup 1.0000x reference)
"""Trainium2 Bass kernel for nn_CompressedInteractionNet_31997506355236.

Reference math (per batch b, channel k, dim d; m == H == 64, D == 16, vk == 16):
    x0r[b,d,:]  = x_0[b,:,d]                      # [m]
    xhr[b,d,:]  = x_0[b].reshape(D, H)[d]         # [H] (flat reinterpretation)
    out[b,k,d]  = sum_v (x0r[b,d] @ Vm[k,0,:,v]) * (Vh[k,0,v,:] @ xhr[b,d])

Strategy: 2D sharding, batch x channels = 4 x 2 over 8 cores (32 batches and
32 output channels per core) — minimizes per-core DMA bytes at equal compute.
Host-side sharding lays the operands out so every device DMA is fully
contiguous (DMA engines are packet/descriptor-rate-bound; strided 64B-run
loads are ~10x slower):
    xc  [m, 2*bd]  = [x0t | xhrt]  (both lhsT operands, per batch shard)
    vmf [m, 512], vhf [j, 512]     (rhs operands, per k shard)
Device, per 128-row chunk c (4 units):
    A = x0t_c.T @ vmf, Bt = xhrt_c.T @ vhf      (PE, f32r, PSUM)
    b_sb = copy(Bt)                             (ACT; DVE allows <=1 PSUM input)
    P = A * b_sb                                (DVE)
    O[bd, k] = sum_v P[bd, k, v]                (GPSIMD half-add + DVE reduce;
                                                 last unit all-DVE)
Output leaves the device as [(b,d), k_loc]; the host unshards and transposes
back to [B, Hk, D].
"""

import numpy as np

import concourse.bass as bass
import concourse.tile as tile
from concourse import bacc, mybir
from concourse.bass_utils import run_bass_kernel_spmd

# Problem constants (hardcoded; kernel must be self-contained).
B, M, D = 128, 64, 16
HK, VK = 64, 16
H = 64
NCORES = 8
SB, SK = 4, 2             # batch shards x channel shards
BL = B // SB              # batches per core = 32
BD = BL * D               # rows per core = 512
KL = HK // SK             # channels per core = 32
KVL = KL * VK             # 512
NCH = BD // 128           # 128-row chunks per core = 4
F32 = mybir.dt.float32
F32R = mybir.dt.float32r

_CACHE = {}


def build_bass():
    nc = bacc.Bacc("TRN2", target_bir_lowering=False, debug=False,
                   num_devices=NCORES, enable_partition_id=False,
                   monotonic_sem_count=0)

    # xc piece p holds [x0t chunks 2p,2p+1 | xhrt chunks 2p,2p+1]
    xc0_d = nc.dram_tensor("xc0", [M, BD], F32, kind="ExternalInput")
    xc1_d = nc.dram_tensor("xc1", [M, BD], F32, kind="ExternalInput")
    vmf_d = nc.dram_tensor("vmf", [M, KVL], F32, kind="ExternalInput")
    vhf_d = nc.dram_tensor("vhf", [H, KVL], F32, kind="ExternalInput")
    out = nc.dram_tensor("out", [BD, KL], F32, kind="ExternalOutput")

    with tile.TileContext(nc) as tc:
        with (
            tc.tile_pool(name="w", bufs=1) as w,
            tc.tile_pool(name="work", bufs=3) as work,
            tc.tile_pool(name="pab", bufs=2, space="PSUM") as pab,
            tc.tile_pool(name="pwarm", bufs=1, space="PSUM") as pwarm,
        ):
            # ---- PE warmup during the load window ----------------------
            # The HAM clock gate keeps an idle PE at ~0.65-1.2 GHz; ~3.5us of
            # sustained activity unlocks 2.4 GHz for the real matmuls.
            wz = w.tile([M, 128], F32)
            nc.gpsimd.memset(wz[:], 0.0)
            pz = pwarm.tile([128, 512], F32, tag="warm")
            for _ in range(10):
                nc.tensor.matmul(pz[:, 0:128], wz[:], wz[:],
                                 start=True, stop=True)

            # ---- contiguous loads spread over the 3 issue queues -------
            vhf = w.tile([H, KVL], F32R)
            nc.sync.dma_start(vhf[:], vhf_d.ap().bitcast(F32R))
            vmf = w.tile([M, KVL], F32R)
            nc.scalar.dma_start(vmf[:], vmf_d.ap().bitcast(F32R))
            xc0 = w.tile([M, BD], F32R)
            nc.sync.dma_start(xc0[:], xc0_d.ap().bitcast(F32R))
            xc1 = w.tile([M, BD], F32R)
            nc.gpsimd.dma_start(xc1[:], xc1_d.ap().bitcast(F32R))
            xcs = [xc0, xc1]

            def unit(c, last):
                xp = xcs[c // 2]
                off = (c % 2) * 128
                psum_b = pab.tile([128, KVL], F32, tag="b")
                nc.tensor.matmul(psum_b[:], xp[:, 256 + off:384 + off], vhf[:],
                                 start=True, stop=True)
                psum_a = pab.tile([128, KVL], F32, tag="a")
                nc.tensor.matmul(psum_a[:], xp[:, off:128 + off], vmf[:],
                                 start=True, stop=True)

                b_sb = work.tile([128, KL, VK], F32, tag="b_sb")
                nc.scalar.copy(b_sb.rearrange("p k v -> p (k v)"), psum_b[:])
                p_sb = work.tile([128, KL, VK], F32, tag="p_sb")
                nc.vector.tensor_mul(
                    out=p_sb.rearrange("p k v -> p (k v)"),
                    in0=psum_a[:],
                    in1=b_sb.rearrange("p k v -> p (k v)"))
                o_sb = work.tile([128, KL], F32, tag="o_sb")
                if last:
                    # shortest tail chain: direct DVE reduce over v=16
                    nc.vector.tensor_reduce(out=o_sb[:], in_=p_sb[:],
                                            axis=mybir.AxisListType.X,
                                            op=mybir.AluOpType.add)
                else:
                    # GPSIMD folds v 16->8, DVE reduces the rest
                    t1 = work.tile([128, KL, VK // 2], F32, tag="t1")
                    nc.gpsimd.tensor_tensor(t1[:], p_sb[:, :, 0:8],
                                            p_sb[:, :, 8:16],
                                            mybir.AluOpType.add)
                    nc.vector.tensor_reduce(out=o_sb[:], in_=t1[:],
                                            axis=mybir.AxisListType.X,
                                            op=mybir.AluOpType.add)
                nc.sync.dma_start(out.ap()[128 * c:128 * (c + 1), :], o_sb[:])

            for c in range(NCH):
                unit(c, last=(c == NCH - 1))

    nc.compile()
    return nc


def run(x_0, x_h, Vm, Vh, **spmd_kwargs):
    x_0 = np.ascontiguousarray(np.asarray(x_0), dtype=np.float32)
    vm = np.asarray(Vm)[:, 0].astype(np.float32)
    vh = np.asarray(Vh)[:, 0].astype(np.float32)

    # Host-side layout prep (part of sharding): all-contiguous device inputs.
    vmf = np.ascontiguousarray(vm.transpose(1, 0, 2).reshape(M, HK * VK))
    vhf = np.ascontiguousarray(vh.transpose(2, 0, 1).reshape(H, HK * VK))

    if "nc" not in _CACHE:
        _CACHE["nc"] = build_bass()
    nc = _CACHE["nc"]

    in_maps = []
    for core in range(NCORES):
        cb, ck = divmod(core, SK)
        shard = x_0[BL * cb:BL * (cb + 1)]                    # [BL, M, D]
        x0t = shard.transpose(1, 0, 2).reshape(M, BD)         # [i, (b,d)]
        xhrt = shard.reshape(BL, D, H).transpose(2, 0, 1).reshape(H, BD)
        xc0 = np.ascontiguousarray(
            np.concatenate([x0t[:, 0:256], xhrt[:, 0:256]], axis=1))
        xc1 = np.ascontiguousarray(
            np.concatenate([x0t[:, 256:512], xhrt[:, 256:512]], axis=1))
        ks = slice(KVL * ck, KVL * (ck + 1))
        in_maps.append({
            "xc0": xc0,
            "xc1": xc1,
            "vmf": np.ascontiguousarray(vmf[:, ks]),
            "vhf": np.ascontiguousarray(vhf[:, ks]),
        })

    res = run_bass_kernel_spmd(nc, in_maps, core_ids=list(range(NCORES)),
                               **spmd_kwargs)
    # Unshard: per-core out is [(b,d), k_loc] -> [BL, D, KL] -> [BL, KL, D]
    full = np.empty((B, HK, D), dtype=np.float32)
    for core in range(NCORES):
        cb, ck = divmod(core, SK)
        o = res.results[core]["out"].reshape(BL, D, KL).transpose(0, 2, 1)
        full[BL * cb:BL * (cb + 1), KL * ck:KL * (ck + 1), :] = o
    return full, res


def kernel(x_0, x_h, Vm, Vh):
    return run(x_0, x_h, Vm, Vh)[0]


if __name__ == "__main__":
    rng = np.random.default_rng(0)
    x_0 = rng.standard_normal((B, M, D)).astype(np.float32)
    x_h = rng.standard_normal((B, H, D)).astype(np.float32)
    Vm = rng.standard_normal((HK, 1, M, VK)).astype(np.float32)
    Vh = rng.standard_normal((HK, 1, VK, H)).astype(np.float32)
    got = kernel(x_0, x_h, Vm, Vh)

    x0r = np.transpose(x_0, (0, 2, 1))
    xhr = x_0.reshape(B, D, H)
    a = np.einsum("bdi,kiv->bkdv", x0r, Vm[:, 0])
    bb = np.einsum("bdj,kvj->bkdv", xhr, Vh[:, 0])
    want = np.einsum("bkdv,bkdv->bkd", a, bb)
    err = np.abs(got - want).max() / np.abs(want).max()
    print("rel err:", err)
